# revision 16
# baseline (speedup 1.0000x reference)
"""Multi-head causal self-attention (B=4, S=2048, E=1024, H=16, Dh=64) on 8
Trainium2 NeuronCores.

Sharding: tensor-parallel over heads — 2 heads per core. Each core computes
q/k/v projections, causal attention and its slice of the output projection
(rows of Wo for its heads); the host sums the 8 partial outputs and adds bo.

Per-core layout strategy (all matmul contractions need the contracted dim on
SBUF partitions):
  - x[b] is transposed once per s-chunk on the PE (fp32 transpose is exact)
    into xT [e, s]; projections then produce qT/kT [head*dh, s] directly
    (lhsT = W [e, hd], rhs = xT) and vT, which is PE-transposed into v [t, dh].
  - scoresT[t, s] = kT.T @ qT per 128-key tile; both heads run concurrently
    on the PE via row tiling (head 0 in array rows 0-63, head 1 in 64-127).
  - softmax needs no partition reduction: exp on ACT (no max subtraction —
    scores are bounded ~|2.5| here), causal mask via gpsimd affine_select,
    denominator from a ones-column appended to v (PV output row 64).
  - PV: out_augT[dh+1, s] += v_aug.T @ expT accumulated over key tiles.
  - normalization folds into the output projection: z_h = outT_h.T @ Wo_h per
    head into separate PSUM banks, combined as z0*recip0 + z1*recip1 with
    per-partition (token) scalars; recip rows are PE-transposed to [s, 1].
Matmuls run in float32r (full PE rate vs 4x slower fp32; ~1.6e-4 rel err).
"""

import numpy as np
import concourse.bass as bass
import concourse.mybir as mybir
import concourse.tile as tile
from concourse.vector_clock import ScopedClock
from concourse.masks import make_identity
from concourse.bass_utils import run_bass_kernel_spmd

F32 = mybir.dt.float32
F32R = mybir.dt.float32r
AF = mybir.ActivationFunctionType
ALU = mybir.AluOpType

B, S, E, H, DH = 4, 2048, 1024, 16, 64
NCORES = 8
HP = 2            # heads per core
SC = 512          # query chunk (columns of scoresT)
NSC = S // SC     # 4 chunks per batch
TT = 128          # key tile
NTT = S // TT     # 16 key tiles per batch
EO = E // 128     # 8 contraction chunks


class SafeTileContext(tile.TileContext):
    """TileContext with the tail drain's sem waits split across multiple
    Drain instructions — walrus here rejects >1 sync wait per instruction."""

    MAX_DRAIN_WAITS = 1

    def _drain_and_barrier(self, tick_clock, wait_clock):
        nc = self.nc
        drain_inst = nc.sync.drain()
        wait_clock.add_sem_waits(
            drain_inst.ins, ScopedClock({None: tick_clock.global_clock})
        )
        si = drain_inst.ins.sync_info
        if si is not None and si.on_wait and len(si.on_wait) > self.MAX_DRAIN_WAITS:
            waits = list(si.on_wait)
            si.on_wait = waits[: self.MAX_DRAIN_WAITS]
            drain_inst.ins.sync_info = si
            for i in range(self.MAX_DRAIN_WAITS, len(waits), self.MAX_DRAIN_WAITS):
                extra = nc.sync.drain()
                extra.ins.sync_info = mybir.SyncInfo(
                    on_wait=waits[i : i + self.MAX_DRAIN_WAITS], on_update=[]
                )
        nc.all_engine_barrier()
        assert self.sems is not None
        popped = nc._tile_sem_poison_stack.pop()
        assert popped is self._sem_poison
        nc.clear_and_free_semaphores(list(self.sems.allocated().values()))
        nc.all_engine_barrier()


def split_sync_waits(nc, maxw=1):
    """Hoist excess sync waits onto same-engine NoOps inserted just before
    the over-limit instruction (this container's walrus allows ~1)."""
    n_split = 0
    for f in nc.m.functions:
        for blk in f.blocks:
            out = []
            for ins in blk.instructions:
                si = ins.sync_info
                if si is not None and si.on_wait and len(si.on_wait) > maxw:
                    waits = list(si.on_wait)
                    extra, keep = waits[:-maxw], waits[-maxw:]
                    for j in range(0, len(extra), maxw):
                        nop = mybir.InstNoOp(
                            name=f"{ins.name}-wsplit{j}", ins=[], outs=[]
                        )
                        nop.engine = ins.engine
                        nop.sync_info = mybir.SyncInfo(
                            on_wait=extra[j : j + maxw], on_update=[]
                        )
                        out.append(nop)
                    si.on_wait = keep
                    ins.sync_info = si
                    n_split += 1
                out.append(ins)
            blk.instructions = out
    return n_split


def build_bass(stage=3, nb=B):
    """stage: 0=x-transposes only, 1=+projections, 2=+attention, 3=full.
    nb: number of batches to emit (debug)."""
    nc = bass.Bass()
    x_d = nc.dram_tensor("x", [B, S, E], F32, kind="ExternalInput")
    wq_d = nc.dram_tensor("wq", [HP, E, DH], F32, kind="ExternalInput")
    wk_d = nc.dram_tensor("wk", [HP, E, DH], F32, kind="ExternalInput")
    wv_d = nc.dram_tensor("wv", [HP, E, DH], F32, kind="ExternalInput")
    bqkv_d = nc.dram_tensor("bqkv", [128, 3], F32, kind="ExternalInput")
    wo_d = nc.dram_tensor("wo", [HP * DH, E], F32, kind="ExternalInput")
    z_d = nc.dram_tensor("zpart", [B, S, E], F32, kind="ExternalOutput")

    with SafeTileContext(nc) as tc:
        with (
            tc.tile_pool(name="const", bufs=1) as constp,
            tc.tile_pool(name="xnat", bufs=3) as xnat_p,
            tc.tile_pool(name="xt", bufs=2) as xt_p,
            tc.tile_pool(name="qkv", bufs=2) as qkv_p,
            tc.tile_pool(name="vpool", bufs=2) as v_p,
            tc.tile_pool(name="vt", bufs=2) as vt_p,
            tc.tile_pool(name="expt", bufs=6) as exp_p,
            tc.tile_pool(name="outt", bufs=4) as out_p,
            tc.tile_pool(name="den", bufs=2) as den_p,
            tc.tile_pool(name="zsb", bufs=4) as z_p,
            tc.tile_pool(name="ps_misc", bufs=2, space="PSUM") as ps_misc,
            tc.tile_pool(name="ps_pz", bufs=2, space="PSUM") as ps_pz,
            tc.tile_pool(name="ps_qk", bufs=2, space="PSUM") as ps_qk,
            tc.tile_pool(name="ps_pv", bufs=2, space="PSUM") as ps_pv,
        ):
            ident = constp.tile([128, 128], F32)
            make_identity(nc, ident)

            # weights: [ei, eo, h*dh] stationary layout, rounded to f32r
            wstage = constp.tile([128, EO, 128], F32)
            w_sbs = []
            for nm, wd in (("wq", wq_d), ("wk", wk_d), ("wv", wv_d)):
                w_sb = constp.tile([128, EO, 128], F32R, name=f"{nm}_sb")
                for h in range(HP):
                    nc.sync.dma_start(
                        wstage[:, :, h * DH : (h + 1) * DH],
                        wd[h].rearrange("(eo ei) d -> ei eo d", ei=128),
                    )
                nc.vector.tensor_copy(out=w_sb, in_=wstage)
                w_sbs.append(w_sb)
            wq_sb, wk_sb, wv_sb = w_sbs

            wo_stage = constp.tile([64, HP, E], F32)
            wo_sb = constp.tile([64, HP, E], F32R)
            for h in range(HP):
                nc.sync.dma_start(wo_stage[:, h, :], wo_d[h * DH : (h + 1) * DH, :])
            nc.vector.tensor_copy(out=wo_sb, in_=wo_stage)

            bias_sb = constp.tile([128, 3], F32)
            nc.sync.dma_start(bias_sb, bqkv_d[:, :])

            ones_sb = constp.tile([128, 1], F32)
            nc.vector.memset(ones_sb, 1.0)

            for b in range(nb):
                qT = qkv_p.tile([128, S], F32R, tag="qT")
                kT = qkv_p.tile([128, S], F32R, tag="kT")
                v_sb = v_p.tile([128, HP, NTT, DH + 1], F32R, tag="v")
                nc.vector.tensor_copy(
                    out=v_sb[:, :, :, DH : DH + 1],
                    in_=ones_sb.to_broadcast([128, HP, NTT, 1]),
                )

                for c in range(NSC):
                    s0 = c * SC
                    # ---- xT for this chunk (PE transpose, exact fp32) ----
                    xt = xt_p.tile([128, EO, SC], F32R, tag="xt")
                    for st in range(SC // 128):
                        xin = xnat_p.tile([128, E], F32, tag="xn")
                        nc.sync.dma_start(
                            xin, x_d[b, s0 + st * 128 : s0 + (st + 1) * 128, :]
                        )
                        for eg in range(2):
                            pst = ps_misc.tile([128, 512], F32, tag="m")
                            for e4 in range(4):
                                eo = eg * 4 + e4
                                nc.tensor.transpose(
                                    pst[:, e4 * 128 : (e4 + 1) * 128],
                                    xin[:, eo * 128 : (eo + 1) * 128],
                                    ident,
                                )
                            nc.vector.tensor_copy(
                                out=xt[
                                    :, eg * 4 : (eg + 1) * 4, st * 128 : (st + 1) * 128
                                ],
                                in_=pst.rearrange("p (a b) -> p a b", a=4),
                            )

                    if stage == 0:
                        nc.sync.dma_start(z_d[b, s0 : s0 + 128, 0:E], xin)
                        continue

                    # ---- projections for this chunk ----
                    if stage == 10:
                        proj_list = (("q", wq_sb),)
                    elif stage == 11:
                        proj_list = (("q", wq_sb), ("k", wk_sb))
                    else:
                        proj_list = (("q", wq_sb), ("k", wk_sb), ("v", wv_sb))
                    for kind, w_sb in proj_list:
                        psp = ps_pz.tile([128, 512], F32, tag="pz")
                        for eo in range(EO):
                            nc.tensor.matmul(
                                psp,
                                lhsT=w_sb[:, eo, :],
                                rhs=xt[:, eo, :],
                                start=(eo == 0),
                                stop=(eo == EO - 1),
                            )
                        col = {"q": 0, "k": 1, "v": 2}[kind]
                        bias_ap = bias_sb[:, col : col + 1]
                        if kind == "q":
                            nc.vector.tensor_scalar_add(
                                qT[:, s0 : s0 + SC], psp, bias_ap
                            )
                        elif kind == "k":
                            nc.vector.tensor_scalar_add(
                                kT[:, s0 : s0 + SC], psp, bias_ap
                            )
                        else:
                            vt = vt_p.tile([128, SC], F32, tag="vt")
                            nc.vector.tensor_scalar_add(vt, psp, bias_ap)
                            if stage == 12:
                                continue
                            # transpose vT [hd2, s] -> [s, hd2] per 128-token
                            # tile (full 128-partition transposes)
                            pstv = ps_misc.tile([128, 512], F32, tag="m")
                            for tl in range(SC // 128):
                                nc.tensor.transpose(
                                    pstv[:, tl * 128 : (tl + 1) * 128],
                                    vt[:, tl * 128 : (tl + 1) * 128],
                                    ident,
                                )
                            if stage == 13:
                                zt13 = z_p.tile([128, 512], F32, tag="z")
                                nc.vector.tensor_copy(out=zt13, in_=pstv)
                                continue
                            pv4 = pstv.rearrange("p (a q b) -> p a q b", a=4, q=HP)
                            for h in range(HP):
                                nc.vector.tensor_copy(
                                    out=v_sb[:, h, c * 4 : (c + 1) * 4, 0:DH],
                                    in_=pv4[:, :, h, :],
                                )

                    if stage in (1, 10, 11, 12, 13):
                        zt = z_p.tile([128, 512], F32, tag="z")
                        nc.vector.tensor_copy(out=zt, in_=qT[:, s0 : s0 + SC])
                        nc.sync.dma_start(z_d[b, s0 : s0 + 128, 0:512], zt)
                        continue

                    # ---- causal attention for this chunk ----
                    nkt = 4 * (c + 1)
                    po = [
                        ps_pv.tile([DH + 1, 512], F32, tag="pv", name=f"pv{h}_{b}_{c}")
                        for h in range(HP)
                    ]
                    for tt in range(nkt):
                        for h in range(HP):
                            pss = ps_qk.tile([128, 512], F32, tag="qk")
                            nc.tensor.matmul(
                                pss,
                                lhsT=kT[h * DH : (h + 1) * DH, tt * TT : (tt + 1) * TT],
                                rhs=qT[h * DH : (h + 1) * DH, s0 : s0 + SC],
                                start=True,
                                stop=True,
                            )
                            et = exp_p.tile([128, SC], F32R, tag="et")
                            nc.scalar.activation(et, pss, AF.Exp, scale=0.125)
                            if tt >= 4 * c:
                                # diagonal tile: zero where t > s
                                j = tt - 4 * c
                                # keep where t <= s: iota = s - t_local - 128j >= 0
                                nc.gpsimd.affine_select(
                                    out=et,
                                    in_=et,
                                    pattern=[[1, SC]],
                                    compare_op=ALU.is_ge,
                                    fill=0.0,
                                    base=-128 * j,
                                    channel_multiplier=-1,
                                )
                            nc.tensor.matmul(
                                po[h],
                                lhsT=v_sb[:, h, tt, :],
                                rhs=et,
                                start=(tt == 0),
                                stop=(tt == nkt - 1),
                            )

                    # ---- denominators: recip row 64, transpose to [s, 1] ----
                    drow = den_p.tile([128, HP, SC], F32, tag="drow")
                    for h in range(HP):
                        nc.vector.reciprocal(
                            drow[DH : DH + 1, h, :], po[h][DH : DH + 1, :]
                        )
                    # full [128,128] transposes of the recip rows (only row 64
                    # holds data; the rest is discarded garbage) — the recips
                    # land in column 64 of each transposed block.
                    den_sb = den_p.tile([128, 8], F32, tag="den")
                    for g in range(2):
                        psd = ps_misc.tile([128, 512], F32, tag="m")
                        for i in range(4):
                            h, st = divmod(g * 4 + i, 4)
                            nc.tensor.transpose(
                                psd[:, i * 128 : (i + 1) * 128],
                                drow[:, h, st * 128 : (st + 1) * 128],
                                ident,
                            )
                        nc.vector.tensor_copy(
                            out=den_sb[:, g * 4 : (g + 1) * 4],
                            in_=psd.rearrange("p (a b) -> p a b", a=4)[:, :, DH],
                        )

                    outT = []
                    for h in range(HP):
                        ot = out_p.tile([DH, SC], F32R, tag="ot", name=f"ot{h}_{b}_{c}")
                        nc.vector.tensor_copy(out=ot, in_=po[h][0:DH, :])
                        outT.append(ot)

                    if stage == 2:
                        zt = z_p.tile([128, 512], F32, tag="z")
                        nc.vector.tensor_copy(out=zt[0:DH, :], in_=outT[0])
                        nc.vector.tensor_copy(out=zt[DH : 2 * DH, :], in_=outT[1])
                        nc.sync.dma_start(z_d[b, s0 : s0 + 128, 0:512], zt)
                        continue

                    # ---- output projection (per head) + combine ----
                    for st in range(SC // 128):
                        for ec in range(E // 512):
                            pz = [
                                ps_pz.tile(
                                    [128, 512], F32, tag="pz", name=f"pz{h}_z"
                                )
                                for h in range(HP)
                            ]
                            for h in range(HP):
                                nc.tensor.matmul(
                                    pz[h],
                                    lhsT=outT[h][:, st * 128 : (st + 1) * 128],
                                    rhs=wo_sb[:, h, ec * 512 : (ec + 1) * 512],
                                    start=True,
                                    stop=True,
                                )
                            zt = z_p.tile([128, 512], F32, tag="z")
                            nc.vector.tensor_scalar_mul(
                                zt, pz[0], den_sb[:, st : st + 1]
                            )
                            nc.vector.scalar_tensor_tensor(
                                out=zt,
                                in0=pz[1],
                                scalar=den_sb[:, 4 + st : 4 + st + 1],
                                in1=zt,
                                op0=ALU.mult,
                                op1=ALU.add,
                            )
                            nc.sync.dma_start(
                                z_d[
                                    b,
                                    s0 + st * 128 : s0 + (st + 1) * 128,
                                    ec * 512 : (ec + 1) * 512,
                                ],
                                zt,
                            )

    split_sync_waits(nc, maxw=1)
    return nc


_NC_CACHE = None


def _get_nc():
    global _NC_CACHE
    if _NC_CACHE is None:
        _NC_CACHE = build_bass()
    return _NC_CACHE


def kernel(x, Wq, Wk, Wv, bq, bk, bv, Wo, bo):
    nc = _get_nc()
    x = np.ascontiguousarray(np.asarray(x, dtype=np.float32))
    in_maps = []
    for c in range(NCORES):
        h0 = HP * c
        bias = np.stack(
            [
                np.asarray(bq[h0 : h0 + HP], np.float32).reshape(HP * DH),
                np.asarray(bk[h0 : h0 + HP], np.float32).reshape(HP * DH),
                np.asarray(bv[h0 : h0 + HP], np.float32).reshape(HP * DH),
            ],
            axis=1,
        )
        in_maps.append(
            {
                "x": x,
                "wq": np.ascontiguousarray(Wq[h0 : h0 + HP], np.float32),
                "wk": np.ascontiguousarray(Wk[h0 : h0 + HP], np.float32),
                "wv": np.ascontiguousarray(Wv[h0 : h0 + HP], np.float32),
                "bqkv": np.ascontiguousarray(bias, np.float32),
                "wo": np.ascontiguousarray(
                    Wo[h0 * DH : (h0 + HP) * DH, :], np.float32
                ),
            }
        )
    res = run_bass_kernel_spmd(nc, in_maps, core_ids=list(range(NCORES)))
    acc = np.zeros((B, S, E), np.float64)
    for r in res.results:
        acc += r["zpart"].astype(np.float64)
    acc += np.asarray(bo, np.float64)[None, None, :]
    return acc.astype(np.float32)
